# revision 58
# baseline (speedup 1.0000x reference)
"""GATv2 (3-layer) + sum-pool + BatchNorm + FC on 8 Trainium2 NeuronCores.

Strategy (sharding_hint: partition nodes & incoming edges across cores):
 - Nodes split into 8 contiguous ranges of N/8. Core c owns edges whose dst
   falls in its range. Edges are dst-sorted and grouped into windows of 128
   destination nodes; per window the incoming edges are packed into tiles of
   128 edge-slots (split by src < N/2 so gather indices fit int16).
 - Per layer: every core builds the full xl = h @ Wl table (replicated
   compute, bf16) plus xr for its own dst range; per edge tile it
   dma_gathers xl[src] rows, gathers xr[dst] rows, computes the GATv2
   logits, exp (softmax without max-subtraction: logits are O(1)), and
   aggregates numerator+denominator with a one-hot segment matmul into
   PSUM per window. Per-node epilogue: divide, mean over heads, +bias,
   leaky_relu. h is AllGathered (bf16) across cores between layers.
 - Final layer: windows also accumulate the per-graph sum-pool via a
   host-built graph one-hot matmul; pooled[64,64] is AllReduced, then BN
   (over graph axis) + FC run replicated on every core.

kernel(**inputs) takes the full reference inputs and returns the full
[64, 32] output.
"""
import sys
import os
from contextlib import ExitStack

sys.path.insert(0, "/opt/trn_rl_repo")

import numpy as np
import ml_dtypes

import concourse.bass as bass
import concourse.bacc as bacc
import concourse.tile as tile
import concourse.mybir as mybir
from concourse.bass_utils import run_bass_kernel_spmd

F32 = mybir.dt.float32
BF16 = mybir.dt.bfloat16
I16 = mybir.dt.int16
AX = mybir.AxisListType
OP = mybir.AluOpType
AF = mybir.ActivationFunctionType

NCORES = 8
WIN = 128          # dst nodes per window
HEADS = 4
BN_EPS = 1e-5

GDT = BF16         # gather-table dtype
GDT_NP = ml_dtypes.bfloat16


# ---------------------------------------------------------------- host prep

def _plan(edge_index, batch, n_nodes, n_graphs):
    """Build the shared window structure + per-core slot arrays."""
    npc = n_nodes // NCORES
    nw = (npc + WIN - 1) // WIN
    nhalf = n_nodes // 2  # src split point (so gather idx fits int16)
    src = np.asarray(edge_index[0], np.int64)
    dst = np.asarray(edge_index[1], np.int64)

    # per (core, window): list of (src, dst_local_in_window) split by src half
    per_cw = [[None] * nw for _ in range(NCORES)]
    core_of = dst // npc
    np.clip(core_of, 0, NCORES - 1, out=core_of)  # dst==n_nodes-? safety
    for c in range(NCORES):
        m = core_of == c
        s_c, d_c = src[m], dst[m] - c * npc
        w_c = d_c // WIN
        order = np.argsort(w_c, kind="stable")
        s_c, d_c, w_c = s_c[order], d_c[order], w_c[order]
        bounds = np.searchsorted(w_c, np.arange(nw + 1))
        for w in range(nw):
            ss = s_c[bounds[w]:bounds[w + 1]]
            dd = d_c[bounds[w]:bounds[w + 1]] % WIN
            ma = ss < nhalf
            per_cw[c][w] = (ss[ma], dd[ma], ss[~ma] - nhalf, dd[~ma])

    tA = [0] * nw
    tB = [0] * nw
    for w in range(nw):
        for c in range(NCORES):
            sa, _, sb, _ = per_cw[c][w]
            tA[w] = max(tA[w], (len(sa) + 127) // 128)
            tB[w] = max(tB[w], (len(sb) + 127) // 128)
    T = [tA[w] + tB[w] for w in range(nw)]
    tiles_tot = sum(T)
    toff = np.cumsum([0] + T)  # tile offset of window w

    # per-core arrays
    idx_src = np.zeros((NCORES, 128, tiles_tot * 8), np.int16)
    idx_dst = np.zeros((NCORES, 128, tiles_tot * 8), np.int16)
    dstw = np.full((NCORES, 128, tiles_tot), 255.0, ml_dtypes.bfloat16)
    for c in range(NCORES):
        for w in range(nw):
            sa, da, sb, db = per_cw[c][w]
            na, nb = tA[w] * 128, tB[w] * 128
            svals = np.zeros(na + nb, np.int16)
            svals[: len(sa)] = sa
            svals[na: na + len(sb)] = sb
            dvals = np.zeros(na + nb, np.int16)
            slotw = np.full(na + nb, 255.0, np.float32)
            slotw[: len(sa)] = da
            dvals[: len(sa)] = da + w * WIN
            slotw[na: na + len(sb)] = db
            dvals[na: na + len(sb)] = db + w * WIN
            t0 = toff[w]
            wrapped = svals.reshape(-1, 16).T  # [16, (na+nb)//16]
            idx_src[c, :, t0 * 8:(t0 + T[w]) * 8] = np.tile(wrapped, (8, 1))
            wrappedd = dvals.reshape(-1, 16).T
            idx_dst[c, :, t0 * 8:(t0 + T[w]) * 8] = np.tile(wrappedd, (8, 1))
            dstw[c, :, t0:t0 + T[w]] = slotw.reshape(-1, 128).T

    # graph one-hot per core: [128, nw, n_graphs]
    batch = np.asarray(batch, np.int64)
    G = np.zeros((NCORES, 128, nw, n_graphs), np.float32)
    for c in range(NCORES):
        for w in range(nw):
            lo = w * WIN
            hi = min(lo + WIN, npc)
            g_ids = batch[c * npc + lo: c * npc + hi]
            G[c, np.arange(hi - lo), w, g_ids] = 1.0

    plan = dict(n_nodes=n_nodes, npc=npc, nw=nw, nhalf=nhalf,
                tA=tA, tB=tB, T=T, toff=toff, tiles_tot=tiles_tot,
                n_graphs=n_graphs)
    arrays = dict(idx_src=idx_src, idx_dst=idx_dst, dstw=dstw, G=G)
    return plan, arrays


# ---------------------------------------------------------------- device

def _build(plan, in_dim, hid, lat):
    n_nodes = plan["n_nodes"]
    npc = plan["npc"]
    nw = plan["nw"]
    ngr = plan["n_graphs"]
    tiles_tot = plan["tiles_tot"]
    HD = HEADS * hid
    n_pad = (n_nodes + 127) // 128 * 128
    npc_pad = nw * WIN
    ntt = n_pad // 128          # xl-table tiles
    nto = npc_pad // 128        # xr-table tiles
    T_max = max(plan["T"])
    nhalf = plan["nhalf"]

    nc = bacc.Bacc(None, num_devices=NCORES)

    x_bfT = nc.declare_dram_parameter("x_bfT", [in_dim, n_pad], BF16, isOutput=False)
    x_ownT = nc.declare_dram_parameter("x_ownT", [in_dim, npc_pad], BF16, isOutput=False)
    idx_src = nc.declare_dram_parameter("idx_src", [128, tiles_tot * 8], I16, isOutput=False)
    idx_dst = nc.declare_dram_parameter("idx_dst", [128, tiles_tot * 8], I16, isOutput=False)
    dstw_in = nc.declare_dram_parameter("dstw", [128, tiles_tot], BF16, isOutput=False)
    g_in = nc.declare_dram_parameter("G", [128, nw, ngr], F32, isOutput=False)
    iota_in = nc.declare_dram_parameter("iota", [128, 128], F32, isOutput=False)
    pid_in = nc.declare_dram_parameter("pid", [128, 1], F32, isOutput=False)
    wl_in = nc.declare_dram_parameter("wl", [in_dim, 3, HD], BF16, isOutput=False)
    wr_in = nc.declare_dram_parameter("wr", [in_dim, 3, HD], BF16, isOutput=False)
    attr_in = nc.declare_dram_parameter("att_rep", [128, 3, HD], BF16, isOutput=False)
    b_in = nc.declare_dram_parameter("b_rep", [128, 3, hid], F32, isOutput=False)
    bng_in = nc.declare_dram_parameter("bn_g", [hid, 1], F32, isOutput=False)
    bnb_in = nc.declare_dram_parameter("bn_b", [hid, 1], F32, isOutput=False)
    fcw_in = nc.declare_dram_parameter("fc_wt", [hid, lat], F32, isOutput=False)
    fcb_in = nc.declare_dram_parameter("fc_b_rep", [ngr, lat], F32, isOutput=False)

    y_out = nc.declare_dram_parameter("y_out", [ngr, lat], F32, isOutput=True)

    DBG = bool(os.environ.get("GAT_DEBUG"))
    NOCC = bool(os.environ.get("GAT_NOCC"))
    if DBG:
        nxl = min(256, n_pad)
        nxr = min(256, npc_pad)
        dbg_xl = nc.declare_dram_parameter("dbg_xl", [nxl, HD], F32, isOutput=True)
        dbg_xr = nc.declare_dram_parameter("dbg_xr", [nxr, HD], F32, isOutput=True)
        dbg_hT = nc.declare_dram_parameter("dbg_hT", [hid, npc], F32, isOutput=True)
        dbg_pool = nc.declare_dram_parameter("dbg_pool", [ngr, hid], F32, isOutput=True)
        dbg_u = nc.declare_dram_parameter("dbg_u", [128, HD], F32, isOutput=True)
        dbg_lg = nc.declare_dram_parameter("dbg_lg", [128, HEADS], F32, isOutput=True)
        dbg_acc = nc.declare_dram_parameter("dbg_acc", [128, HD + HEADS], F32, isOutput=True)
        dbg_hsum = nc.declare_dram_parameter("dbg_hsum", [128, hid], F32, isOutput=True)

    xl_tab = nc.dram_tensor("xl_tab", [n_pad, HD], GDT)
    xr_tab = nc.dram_tensor("xr_tab", [npc_pad, HD], GDT)
    h_ownT = nc.dram_tensor("h_ownT", [hid, npc], BF16)
    h_fullT = nc.dram_tensor("h_fullT", [NCORES * hid, npc], BF16,
                             addr_space="Shared")
    pool_in = nc.dram_tensor("pool_in", [ngr, hid], F32)
    pool_red = nc.dram_tensor("pool_red", [ngr, hid], F32, addr_space="Shared")

    with tile.TileContext(nc) as tc, ExitStack() as ctx:
        cpool = ctx.enter_context(tc.tile_pool(name="const", bufs=1))
        tbl = ctx.enter_context(tc.tile_pool(name="tbl", bufs=2))
        gat = ctx.enter_context(tc.tile_pool(name="gat", bufs=2))
        wrk = ctx.enter_context(tc.tile_pool(name="wrk", bufs=2))
        fin = ctx.enter_context(tc.tile_pool(name="fin", bufs=2))
        pp = ctx.enter_context(tc.tile_pool(name="psum", bufs=2, space="PSUM"))
        ppw = ctx.enter_context(tc.tile_pool(name="psumw", bufs=2, space="PSUM"))
        pp1 = ctx.enter_context(tc.tile_pool(name="psum1", bufs=1, space="PSUM"))

        # ---- resident constants
        isrc = cpool.tile([128, tiles_tot * 8], I16)
        nc.sync.dma_start(isrc[:], idx_src[:])
        idst = cpool.tile([128, tiles_tot * 8], I16)
        nc.sync.dma_start(idst[:], idx_dst[:])
        dstw = cpool.tile([128, tiles_tot], BF16)
        nc.sync.dma_start(dstw[:], dstw_in[:])
        iota = cpool.tile([128, 128], F32)
        nc.sync.dma_start(iota[:], iota_in[:])
        iota_bf = cpool.tile([128, 128], BF16)
        nc.vector.tensor_copy(iota_bf[:], iota[:])
        pid = cpool.tile([128, 1], F32)
        nc.sync.dma_start(pid[:], pid_in[:])
        wl_sb = cpool.tile([in_dim, 3, HD], BF16)
        nc.sync.dma_start(wl_sb[:], wl_in[:])
        wr_sb = cpool.tile([in_dim, 3, HD], BF16)
        nc.sync.dma_start(wr_sb[:], wr_in[:])
        attr = cpool.tile([128, 3, HD], BF16)
        nc.sync.dma_start(attr[:], attr_in[:])
        b_sb = cpool.tile([128, 3, hid], F32)
        nc.sync.dma_start(b_sb[:], b_in[:])
        g_sb = cpool.tile([128, nw, ngr], F32)
        nc.sync.dma_start(g_sb[:], g_in[:])

        toff = plan["toff"]
        # f32 identity [128,128] built from iota: ident[p, s] = (s == p)
        ident128 = cpool.tile([128, 128], F32)
        nc.vector.tensor_scalar(ident128[:], iota[:], pid[:], None,
                                OP.is_equal, OP.bypass)

        GS = 8          # table tiles per write group
        STW = nto * 128  # staging width

        def table_block(loader, blk_rows, dst_t, dst_row0, w_t, din, layer):
            """Build table rows [dst_row0, dst_row0+blk_rows) from one staged
            transposed source block. Few big DMAs instead of per-tile ones."""
            st = tbl.tile([din, STW], BF16, tag="src_st")
            loader(st)
            n_t = (blk_rows + 127) // 128
            for g in range(0, n_t, GS):
                t1g = min(g + GS, n_t)
                ot = tbl.tile([128, GS, HD], GDT, tag="tabout")
                n_full = 0
                for t in range(g, t1g):
                    cols = min(128, blk_rows - t * 128)
                    ps = pp.tile([128, HD], F32, tag="tabps")
                    nc.tensor.matmul(ps[0:cols, :],
                                     st[:, t * 128:t * 128 + cols],
                                     w_t[0:din, layer, :],
                                     start=True, stop=True)
                    nc.scalar.activation(ot[0:cols, t - g, :], ps[0:cols, :],
                                         AF.Copy)
                    if cols == 128:
                        n_full += 1
                r0 = dst_row0 + g * 128
                if n_full:
                    dst_ap = dst_t[r0:r0 + n_full * 128, :].rearrange(
                        "(a p) d -> p a d", p=128)
                    nc.sync.dma_start(dst_ap, ot[:, 0:n_full, :])
                if n_full < t1g - g:  # trailing partial tile
                    cols = blk_rows - (g + n_full) * 128
                    nc.sync.dma_start(
                        dst_t[r0 + n_full * 128:r0 + n_full * 128 + cols, :],
                        ot[0:cols, n_full, :])

        def build_tables(layer, din):
            if layer == 0:
                # xl from x_bfT: contiguous chunks of STW cols (all full tiles)
                c0 = 0
                while c0 < n_pad:
                    c1 = min(c0 + STW, n_pad)

                    def ld(st, c0=c0, c1=c1):
                        nc.sync.dma_start(st[:, 0:c1 - c0], x_bfT[:, c0:c1])

                    table_block(ld, c1 - c0, xl_tab, c0, wl_sb, din, layer)
                    c0 = c1

                def ldo(st):
                    nc.sync.dma_start(st[:], x_ownT[:])

                table_block(ldo, npc_pad, xr_tab, 0, wr_sb, din, layer)
            else:
                for b in range(NCORES):
                    def ld(st, b=b):
                        nc.sync.dma_start(st[0:hid, 0:npc],
                                          h_fullT[b * hid:(b + 1) * hid, :])

                    table_block(ld, npc, xl_tab, b * npc, wl_sb, din, layer)

                def ldo(st):
                    nc.sync.dma_start(st[0:hid, 0:npc], h_ownT[:])

                table_block(ldo, npc, xr_tab, 0, wr_sb, din, layer)

        for layer in range(3):
            din = in_dim if layer == 0 else hid
            build_tables(layer, din)

            if DBG and layer == 0:
                nc.gpsimd.dma_start(dbg_xl[:], xl_tab[0:nxl, :])
                nc.gpsimd.dma_start(dbg_xr[:], xr_tab[0:nxr, :])

            # ---- edge phase
            pool_ps = None
            if layer == 2:
                pool_ps = pp1.tile([ngr, hid], F32, tag="pool", name="pool_ps")
            for w in range(nw):
                tAw, tBw, Tw = plan["tA"][w], plan["tB"][w], plan["T"][w]
                t0 = toff[w]
                xl_g = gat.tile([128, Tw, HD], GDT, tag="xl_g")
                xr_g = gat.tile([128, Tw, HD], GDT, tag="xr_g")
                # gathers: A half, B half, dst (split to <=512 idx per call
                # to bound SWDGE descriptor bursts)
                GMAX = 4

                def gath(dst_tile, dt0, n_t, tab, idx_t, col0):
                    for s in range(0, n_t, GMAX):
                        k = min(GMAX, n_t - s)
                        nc.gpsimd.dma_gather(
                            out_ap=dst_tile[:, dt0 + s:dt0 + s + k, :],
                            in_ap=tab,
                            idxs_ap=idx_t[:, (col0 + s) * 8:(col0 + s + k) * 8],
                            num_idxs=k * 128, num_idxs_reg=k * 128,
                            elem_size=HD)

                if tAw:
                    gath(xl_g, 0, tAw, xl_tab[0:nhalf, :], isrc, t0)
                if tBw:
                    gath(xl_g, tAw, tBw, xl_tab[nhalf:n_pad, :], isrc, t0 + tAw)
                gath(xr_g, 0, Tw, xr_tab[:], idst, t0)

                # u = xl + xr ; w = lrelu(u, 0.2) ; v = w * att ; l = sum_d v
                u = wrk.tile([128, Tw, HD], BF16, tag="u")
                nc.vector.tensor_tensor(u[:], xl_g[:], xr_g[:], OP.add)
                t2 = wrk.tile([128, Tw, HD], BF16, tag="t2")
                nc.vector.tensor_scalar_mul(t2[:], u[:], 0.2)
                nc.vector.tensor_tensor(u[:], u[:], t2[:], OP.max)
                a_ap = attr[:, layer, :]
                a_b = bass.AP(a_ap.tensor, a_ap.offset,
                              [a_ap.ap[0], [0, Tw], a_ap.ap[1]])
                nc.vector.tensor_tensor(t2[:], u[:], a_b, OP.mult)
                # logits: tree-fold 64 -> 16, then reduce (reduce is 1x-only)
                v4 = t2[:].rearrange("p t (h d) -> p t h d", h=HEADS)
                f32v = u[:, :, 0:HD // 2].rearrange("p t (h d) -> p t h d", h=HEADS)
                nc.vector.tensor_tensor(f32v, v4[:, :, :, 0:hid // 2],
                                        v4[:, :, :, hid // 2:hid], OP.add)
                f16v = u[:, :, HD // 2:HD * 3 // 4].rearrange(
                    "p t (h d) -> p t h d", h=HEADS)
                nc.vector.tensor_tensor(f16v, f32v[:, :, :, 0:hid // 4],
                                        f32v[:, :, :, hid // 4:hid // 2], OP.add)
                lg = wrk.tile([128, Tw, HEADS], F32, tag="lg")
                nc.vector.tensor_reduce(lg[:], f16v, AX.X, OP.add)

                # rhs = [exp*xl | exp]
                rhs = wrk.tile([128, Tw, HD + HEADS], BF16, tag="rhs")
                nc.scalar.activation(rhs[:, :, HD:HD + HEADS], lg[:], AF.Exp)
                e_ap = rhs[:, :, HD:HD + HEADS]
                e_b = bass.AP(e_ap.tensor, e_ap.offset,
                              [e_ap.ap[0], e_ap.ap[1], e_ap.ap[2], [0, hid]])
                nc.vector.tensor_tensor(
                    rhs[:, :, 0:HD].rearrange("p t (h d) -> p t h d", h=HEADS),
                    xl_g[:].rearrange("p t (h d) -> p t h d", h=HEADS),
                    e_b, OP.mult)

                # seg one-hot + aggregation matmuls
                seg = wrk.tile([128, Tw, 128], BF16, tag="seg")
                d_ap = dstw[:, t0:t0 + Tw]
                d_b = bass.AP(d_ap.tensor, d_ap.offset,
                              [d_ap.ap[0], d_ap.ap[1], [0, 128]])
                i_ap = iota_bf[:]
                i_b = bass.AP(i_ap.tensor, i_ap.offset,
                              [i_ap.ap[0], [0, Tw], i_ap.ap[1]])
                nc.vector.tensor_tensor(seg[:], d_b, i_b, OP.is_equal)

                acc = ppw.tile([128, HD + HEADS], F32, tag="acc")
                for t in range(Tw):
                    nc.tensor.matmul(acc[:], seg[:, t, :], rhs[:, t, :],
                                     start=(t == 0), stop=(t == Tw - 1))

                if DBG and layer == 0 and w == 0:
                    nc.gpsimd.dma_start(dbg_u[:], u[:, 0, :])
                    nc.gpsimd.dma_start(dbg_lg[:], lg[:, 0, :])
                    dbg_acc_sb = fin.tile([128, HD + HEADS], F32, tag="dbga")
                    nc.vector.tensor_copy(dbg_acc_sb[:], acc[:])
                    nc.sync.dma_start(dbg_acc[:], dbg_acc_sb[:])

                # ---- per-node epilogue for this window
                den = fin.tile([128, HEADS], F32, tag="den")
                # clamp instead of +1e-16: keeps reciprocal in range for
                # zero-edge nodes (numerator is 0 there anyway)
                nc.vector.tensor_scalar_max(den[:], acc[:, HD:HD + HEADS], 1e-10)
                rd = fin.tile([128, HEADS], F32, tag="rd")
                nc.vector.reciprocal(rd[:], den[:])
                nc.vector.tensor_scalar_mul(rd[:], rd[:], 1.0 / HEADS)
                avg = fin.tile([128, HEADS, hid], F32, tag="avg")
                r_b = bass.AP(rd[:].tensor, rd[:].offset,
                              [rd[:].ap[0], rd[:].ap[1], [0, hid]])
                nc.vector.tensor_tensor(
                    avg[:], acc[:, 0:HD].rearrange("p (h d) -> p h d", h=HEADS),
                    r_b, OP.mult)
                hsum = fin.tile([128, hid], F32, tag="hsum")
                nc.vector.tensor_reduce(
                    hsum[:], avg[:].rearrange("p h d -> p d h"), AX.X, OP.add)
                nc.vector.tensor_tensor(hsum[:], hsum[:], b_sb[:, layer, :], OP.add)
                h01 = fin.tile([128, hid], F32, tag="h01")
                nc.vector.tensor_scalar_mul(h01[:], hsum[:], 0.1)
                nc.vector.tensor_tensor(hsum[:], hsum[:], h01[:], OP.max)

                if DBG and layer == 0 and w == 0:
                    nc.sync.dma_start(dbg_hsum[:], hsum[:])
                if layer < 2:
                    htp = pp1.tile([hid, 128], F32, tag="htp")
                    nc.tensor.transpose(htp[:], hsum[:], ident128[:])
                    hT = fin.tile([hid, 128], BF16, tag="hT")
                    nc.scalar.activation(hT[:], htp[:], AF.Copy)
                    vw = min(WIN, npc - w * WIN)
                    nc.sync.dma_start(h_ownT[:, w * WIN:w * WIN + vw],
                                      hT[:, 0:vw])
                else:
                    nc.tensor.matmul(pool_ps[:], g_sb[:, w, :], hsum[:],
                                     start=(w == 0), stop=(w == nw - 1))
                    if w == nw - 1:
                        pool_sb = fin.tile([ngr, hid], F32, tag="pool_sb")
                        nc.vector.tensor_copy(pool_sb[:], pool_ps[:])
                        nc.sync.dma_start(pool_in[:], pool_sb[:])

            if DBG and layer == 0:
                nc.gpsimd.dma_start(dbg_hT[:], h_ownT[:])
            if layer < 2:
                if NOCC:
                    for c in range(NCORES):
                        nc.sync.dma_start(h_fullT[c * hid:(c + 1) * hid, :],
                                          h_ownT[:])
                else:
                    nc.gpsimd.collective_compute(
                        "AllGather", OP.bypass,
                        replica_groups=[list(range(NCORES))],
                        ins=[h_ownT[:]],
                        outs=[h_fullT[:]])

        # ---- pooled AllReduce + BN + FC (replicated)
        if NOCC:
            nc.sync.dma_start(pool_red[:], pool_in[:])
        else:
            nc.gpsimd.collective_compute(
                "AllReduce", OP.add,
                replica_groups=[list(range(NCORES))],
                ins=[pool_in[:]], outs=[pool_red[:]])

        pool2 = fin.tile([ngr, hid], F32, tag="pool2")
        nc.sync.dma_start(pool2[:], pool_red[:])
        if DBG:
            nc.sync.dma_start(dbg_pool[:], pool_red[:])
        ptp = pp1.tile([hid, ngr], F32, tag="ptp")
        nc.tensor.transpose(ptp[:], pool2[:], ident128[0:ngr, 0:ngr])
        pt = fin.tile([hid, ngr], F32, tag="pt")
        nc.vector.tensor_copy(pt[:], ptp[:])

        mean = fin.tile([hid, 1], F32, tag="mean")
        nc.vector.tensor_reduce(mean[:], pt[:], AX.X, OP.add)
        nc.vector.tensor_scalar_mul(mean[:], mean[:], 1.0 / ngr)
        sq = fin.tile([hid, ngr], F32, tag="sq")
        nc.vector.tensor_tensor(sq[:], pt[:], pt[:], OP.mult)
        var = fin.tile([hid, 1], F32, tag="var")
        nc.vector.tensor_reduce(var[:], sq[:], AX.X, OP.add)
        nc.vector.tensor_scalar_mul(var[:], var[:], 1.0 / ngr)
        m2 = fin.tile([hid, 1], F32, tag="m2")
        nc.vector.tensor_tensor(m2[:], mean[:], mean[:], OP.mult)
        nc.vector.tensor_tensor(var[:], var[:], m2[:], OP.subtract)
        nc.vector.tensor_scalar_add(var[:], var[:], BN_EPS)
        std = fin.tile([hid, 1], F32, tag="std")
        nc.scalar.activation(std[:], var[:], AF.Sqrt)
        rstd = fin.tile([hid, 1], F32, tag="rstd")
        nc.vector.reciprocal(rstd[:], std[:])

        bng = fin.tile([hid, 1], F32, tag="bng")
        nc.sync.dma_start(bng[:], bng_in[:])
        bnb = fin.tile([hid, 1], F32, tag="bnb")
        nc.sync.dma_start(bnb[:], bnb_in[:])
        yt = fin.tile([hid, ngr], F32, tag="yt")
        nc.vector.tensor_scalar(yt[:], pt[:], mean[:], rstd[:],
                                OP.subtract, OP.mult)
        nc.vector.tensor_scalar(yt[:], yt[:], bng[:], bnb[:],
                                OP.mult, OP.add)

        fcw = fin.tile([hid, lat], F32, tag="fcw")
        nc.sync.dma_start(fcw[:], fcw_in[:])
        ops = pp1.tile([ngr, lat], F32, tag="ops")
        nc.tensor.matmul(ops[:], yt[:], fcw[:], start=True, stop=True)
        fcb = fin.tile([ngr, lat], F32, tag="fcb")
        nc.sync.dma_start(fcb[:], fcb_in[:])
        yo = fin.tile([ngr, lat], F32, tag="yo")
        nc.vector.tensor_tensor(yo[:], ops[:], fcb[:], OP.add)
        nc.sync.dma_start(y_out[:], yo[:])

    nc.compile()
    return nc


# ---------------------------------------------------------------- entry

def _make_in_maps(inputs, plan, arrs, in_dim, hid, lat, n_graphs):
    x = np.asarray(inputs["x"], np.float32)
    n_nodes = x.shape[0]
    HD = HEADS * hid
    npc = plan["npc"]
    n_pad = (n_nodes + 127) // 128 * 128
    npc_pad = plan["nw"] * WIN

    x_bfT = np.zeros((in_dim, n_pad), GDT_NP)
    x_bfT[:, :n_nodes] = x.T.astype(GDT_NP)

    wl = np.zeros((in_dim, 3, HD), GDT_NP)
    wr = np.zeros((in_dim, 3, HD), GDT_NP)
    attr = np.zeros((128, 3, HD), ml_dtypes.bfloat16)
    brep = np.zeros((128, 3, hid), np.float32)
    for l in range(3):
        din = in_dim if l == 0 else hid
        wl[0:din, l, :] = np.asarray(inputs[f"Wl{l}"], np.float32).astype(GDT_NP)
        wr[0:din, l, :] = np.asarray(inputs[f"Wr{l}"], np.float32).astype(GDT_NP)
        attr[:, l, :] = np.asarray(inputs[f"att{l}"], np.float32).reshape(-1)[
            None, :].astype(ml_dtypes.bfloat16)
        brep[:, l, :] = np.asarray(inputs[f"b{l}"], np.float32)[None, :]

    iota = np.tile(np.arange(128, dtype=np.float32)[None, :], (128, 1))
    pid = np.arange(128, dtype=np.float32).reshape(128, 1)
    bng = np.asarray(inputs["bn_gamma"], np.float32).reshape(hid, 1)
    bnb = np.asarray(inputs["bn_beta"], np.float32).reshape(hid, 1)
    fcwt = np.asarray(inputs["fc_W"], np.float32).T.copy()      # [hid, lat]
    fcb = np.tile(np.asarray(inputs["fc_b"], np.float32)[None, :], (n_graphs, 1))

    in_maps = []
    for c in range(NCORES):
        x_ownT = np.zeros((in_dim, npc_pad), GDT_NP)
        x_ownT[:, 0:npc] = x[c * npc:(c + 1) * npc].T.astype(GDT_NP)
        in_maps.append(dict(
            x_bfT=x_bfT, x_ownT=x_ownT,
            idx_src=arrs["idx_src"][c], idx_dst=arrs["idx_dst"][c],
            dstw=arrs["dstw"][c], G=arrs["G"][c],
            iota=iota, pid=pid, wl=wl, wr=wr, att_rep=attr, b_rep=brep,
            bn_g=bng, bn_b=bnb, fc_wt=fcwt, fc_b_rep=fcb,
        ))
    return in_maps


def kernel(**inputs):
    x = np.asarray(inputs["x"], np.float32)
    edge_index = np.asarray(inputs["edge_index"])
    batch = np.asarray(inputs["batch"])
    n_nodes, in_dim = x.shape
    hid = inputs["att0"].shape[1]
    lat = inputs["fc_W"].shape[0]
    n_graphs = int(os.environ.get("GAT_NGRAPHS", "64"))

    plan, arrs = _plan(edge_index, batch, n_nodes, n_graphs)
    nc = _build(plan, in_dim, hid, lat)
    in_maps = _make_in_maps(inputs, plan, arrs, in_dim, hid, lat, n_graphs)

    res = None
    last_exc = None
    for attempt in range(3):
        try:
            res = run_bass_kernel_spmd(nc, in_maps, list(range(NCORES)))
            break
        except Exception as e:  # flaky NRT exec-unit errors: retry
            last_exc = e
            import time as _t
            _t.sleep(5)
    if res is None:
        raise last_exc
    global LAST_RESULTS, LAST_PLAN, LAST_ARRS
    LAST_RESULTS, LAST_PLAN, LAST_ARRS = res.results, plan, arrs
    return res.results[0]["y_out"].astype(np.float32)


LAST_RESULTS = None
LAST_PLAN = None
LAST_ARRS = None


if __name__ == "__main__":
    sys.path.insert(0, os.path.dirname(os.path.abspath(__file__)))
    import jax
    import reference
    with jax.default_device(jax.devices("cpu")[0]):
        inputs = {k: np.asarray(v) for k, v in reference.setup_inputs().items()}
        expected = np.asarray(reference.reference(**inputs))
    actual = kernel(**inputs)
    err = np.abs(actual - expected).max() / (np.abs(expected).max() + 1e-12)
    print("Relative error:", err)
